# revision 2
# baseline (speedup 1.0000x reference)
"""MinibatchDiscrimination Trainium2 kernel (8 NeuronCores).

Reference computation:
    m = (x @ T.reshape(F, O*K)).reshape(N, O, K)          # N=512, F=512, O=128, K=8
    d[i,j,o]  = sum_k |m[j,o,k] - m[i,o,k]|
    feats[i,o] = sum_j exp(-d[i,j,o])
    out = concat([x, feats], axis=1)                      # [N, F+O]

Distribution: every core holds the full projected matrix m^T (built on-device
from replicated x^T and T), and computes feats for its own 64 rows of x
(row-sharding over the i index of the pairwise tensor). No collectives needed.

Per-core dataflow (partitions = 32 o-values x 4 k-values per tile):
  - TensorE builds m^T tiles (bf16) and the per-row scalar columns from the
    same matmuls, so the self-pair distance is exactly zero.
  - |m[j,:] - m[i,:]| tiles: split between VectorE (tensor_scalar subtract
    + sign-bit AND abs, batched) and ScalarE (fused Abs(x + bias)).
  - k-reduction: TensorE matmul against a 0/1 selector stationary, PSUM accum.
  - exp(-d) + j-sum: ScalarE activation with accum_out.
"""

import os
import sys
import types
import numpy as np
import ml_dtypes

N, F, O, K = 512, 512, 128, 8
NCORES = 8
ROWS = N // NCORES            # 64 i-rows per core
NG = 4                        # o-groups of 32
NH = 2                        # k-halves of 4
SPLIT = 44                    # i_loc < SPLIT -> VectorE path, else ScalarE path
assert SPLIT % 4 == 0

_CACHE = {}


def _install_axon_shim():
    """Register the NTFF profile hook module that concourse expects under axon."""
    if 'antenv.axon_hooks' in sys.modules:
        return
    try:
        import antenv
    except ImportError:
        return
    mod = types.ModuleType('antenv.axon_hooks')
    mod._hook = None
    mod.set_axon_ntff_profile_hook = lambda h: setattr(mod, '_hook', h)
    mod.get_axon_ntff_profile_hook = lambda: mod._hook
    sys.modules['antenv.axon_hooks'] = mod
    antenv.axon_hooks = mod
    try:
        from trn_agent_boot.trn_boot import _ntff_profile_via_ctypes
        mod.set_axon_ntff_profile_hook(
            _ntff_profile_via_ctypes('/opt/axon/libaxon_pjrt.so'))
    except Exception:
        pass
    import concourse.bass_utils as bu
    bu.upload_artifacts = lambda tmpdir: tmpdir


def _col_perm():
    """Permutation of T2 columns: new column (g*NH+h)*128 + o_l*4 + k_l maps to
    original column (32g + o_l)*K + 4h + k_l."""
    cols = np.empty(O * K, dtype=np.int64)
    idx = 0
    for g in range(NG):
        for h in range(NH):
            for o_l in range(32):
                for k_l in range(4):
                    cols[idx] = (32 * g + o_l) * K + 4 * h + k_l
                    idx += 1
    return cols


def _build_nc():
    from concourse import bass, mybir, bacc
    from concourse import tile

    dt = mybir.dt
    AF = mybir.ActivationFunctionType
    OP = mybir.AluOpType

    nc = bacc.Bacc("TRN2", target_bir_lowering=False, debug=False)

    xT_d = nc.dram_tensor("xT", [F, N], dt.bfloat16, kind="ExternalInput")
    t2_d = nc.dram_tensor("T2p", [F, O * K], dt.bfloat16, kind="ExternalInput")
    xr_d = nc.dram_tensor("xr", [F, ROWS], dt.bfloat16, kind="ExternalInput")
    sel_d = nc.dram_tensor("sel", [128, 32], dt.bfloat16, kind="ExternalInput")
    out_d = nc.dram_tensor("feats", [128, ROWS], dt.float32, kind="ExternalOutput")

    with tile.TileContext(nc) as tc:
        with tc.tile_pool(name="const", bufs=1) as cp, \
             tc.tile_pool(name="work", bufs=3) as wp, \
             tc.tile_pool(name="escr", bufs=2) as ep, \
             tc.tile_pool(name="pbuild", bufs=2, space="PSUM") as pb, \
             tc.tile_pool(name="pd", bufs=4, space="PSUM") as pdp:

            xt = [cp.tile([128, N], dt.bfloat16, tag=f"xt{c}", name=f"xt{c}") for c in range(4)]
            t2 = [cp.tile([128, O * K], dt.bfloat16, tag=f"t2{c}", name=f"t2{c}") for c in range(4)]
            xr = [cp.tile([128, ROWS], dt.bfloat16, tag=f"xr{c}", name=f"xr{c}") for c in range(4)]
            sel = cp.tile([128, 32], dt.bfloat16, tag="sel")
            mt = cp.tile([128, NG * NH * N], dt.bfloat16, tag="mt")       # 8 tiles of [128,512]
            mrb = cp.tile([128, NG * NH * ROWS], dt.bfloat16, tag="mrb")  # bf16-rounded row scalars
            mrf = cp.tile([128, NG * NH * ROWS], dt.float32, tag="mrf")   # +f32 for VectorE
            mrn = cp.tile([128, NG * NH * ROWS], dt.float32, tag="mrn")   # -f32 for ScalarE bias
            feats = cp.tile([128, ROWS], dt.float32, tag="feats")

            for c in range(4):
                nc.sync.dma_start(xt[c][:], xT_d[128 * c:128 * (c + 1), :])
                nc.sync.dma_start(t2[c][:], t2_d[128 * c:128 * (c + 1), :])
                nc.sync.dma_start(xr[c][:], xr_d[128 * c:128 * (c + 1), :])
            nc.sync.dma_start(sel[:], sel_d[:])

            # ---- build m^T tiles (one per (g,h)) and row-scalar columns ----
            pr = pb.tile([128, NG * NH * ROWS], dt.float32, tag="pr")
            for u in range(NG * NH):
                pm = pb.tile([128, N], dt.float32, tag="pm")
                for c in range(4):
                    lhsT = t2[c][:, 128 * u:128 * (u + 1)]
                    nc.tensor.matmul(pm[:], lhsT, xt[c][:],
                                     start=(c == 0), stop=(c == 3))
                nc.scalar.copy(mt[:, N * u:N * (u + 1)], pm[:])
                for c in range(4):
                    lhsT = t2[c][:, 128 * u:128 * (u + 1)]
                    nc.tensor.matmul(pr[:, ROWS * u:ROWS * (u + 1)], lhsT, xr[c][:],
                                     start=(c == 0), stop=(c == 3))
            nc.scalar.copy(mrb[:], pr[:])
            nc.vector.tensor_copy(mrf[:], mrb[:])          # bf16 -> f32 upcast
            nc.scalar.mul(mrn[:], mrf[:], -1.0)

            # ---- main loop ----
            for g in range(NG):
                for b in range(ROWS // 4):                 # i-batches of 4
                    cbig = wp.tile([128, 8 * N], dt.bfloat16, tag="cbig")
                    dve_batch = (4 * b + 3) < SPLIT
                    for q in range(4):
                        i_loc = 4 * b + q
                        for h in range(NH):
                            u = g * NH + h
                            msl = mt[:, N * u:N * (u + 1)]
                            dst = cbig[:, (q * NH + h) * N:(q * NH + h + 1) * N]
                            if dve_batch:
                                nc.vector.tensor_scalar(
                                    dst, msl,
                                    mrf[:, ROWS * u + i_loc:ROWS * u + i_loc + 1],
                                    None, OP.subtract)
                            else:
                                nc.scalar.activation(
                                    dst, msl, AF.Abs,
                                    bias=mrn[:, ROWS * u + i_loc:ROWS * u + i_loc + 1],
                                    scale=1.0)
                    if dve_batch:
                        cu = cbig[:].bitcast(mybir.dt.uint16)
                        nc.vector.tensor_scalar(cu, cu, 0x7FFF, None, OP.bitwise_and)
                    pd = pdp.tile([128, N], dt.float32, tag="pd")
                    for q in range(4):
                        for h in range(NH):
                            nc.tensor.matmul(
                                pd[32 * q:32 * (q + 1), :], sel[:],
                                cbig[:, (q * NH + h) * N:(q * NH + h + 1) * N],
                                start=(h == 0), stop=(h == 1),
                                tile_position=(0, 32 * q))
                    e = ep.tile([128, N], dt.bfloat16, tag="e")
                    nc.scalar.activation(e[:], pd[:], AF.Exp, scale=-1.0,
                                         accum_out=feats[:, g * 16 + b:g * 16 + b + 1])

            nc.sync.dma_start(out_d[:], feats[:])

    nc.compile()
    return nc


def _get_compiled():
    if 'nc' not in _CACHE:
        _install_axon_shim()
        _CACHE['nc'] = _build_nc()
        _CACHE['perm'] = _col_perm()
    return _CACHE['nc'], _CACHE['perm']


def kernel(x: np.ndarray, T: np.ndarray) -> np.ndarray:
    from concourse.bass_utils import run_bass_kernel_spmd

    nc, perm = _get_compiled()

    bf = ml_dtypes.bfloat16
    xT = np.ascontiguousarray(x.T).astype(bf)                       # [F, N]
    t2p = np.ascontiguousarray(T.reshape(F, O * K)[:, perm]).astype(bf)
    selv = (np.arange(128)[:, None] // 4 == np.arange(32)[None, :]).astype(bf)

    in_maps = []
    for c in range(NCORES):
        xr = np.ascontiguousarray(xT[:, ROWS * c:ROWS * (c + 1)])
        in_maps.append({"xT": xT, "T2p": t2p, "xr": xr, "sel": selv})

    trace = bool(int(os.environ.get("MBD_TRACE", "0")))
    res = run_bass_kernel_spmd(nc, in_maps, list(range(NCORES)), trace=trace)
    globals()['LAST_EXEC_NS'] = res.exec_time_ns

    feats = np.empty((N, O), dtype=np.float32)
    for c in range(NCORES):
        fr = res.results[c]["feats"]                                 # [128, 64]
        # partition p = q*32 + o_l ; column = g*16 + b ; i_loc = 4*b + q
        blk = fr.reshape(4, 32, NG, 16).transpose(3, 0, 2, 1).reshape(ROWS, O)
        feats[ROWS * c:ROWS * (c + 1), :] = blk
    return np.concatenate([x.astype(np.float32), feats], axis=1)


# revision 3
# speedup vs baseline: 1.1565x; 1.1565x over previous
"""MinibatchDiscrimination Trainium2 kernel (8 NeuronCores).

Reference computation:
    m = (x @ T.reshape(F, O*K)).reshape(N, O, K)          # N=512, F=512, O=128, K=8
    d[i,j,o]  = sum_k |m[j,o,k] - m[i,o,k]|
    feats[i,o] = sum_j exp(-d[i,j,o])
    out = concat([x, feats], axis=1)                      # [N, F+O]

Distribution: every core holds the full projected matrix m^T (built on-device
from replicated x^T and T), and computes feats for its own 64 rows of x
(row-sharding over the i index of the pairwise tensor). No collectives needed.

Per-core dataflow (partitions = 32 o-values x 4 k-values per tile):
  - TensorE builds m^T tiles (bf16) and the per-row scalar columns from the
    same matmuls, so the self-pair distance is exactly zero.
  - |m[j,:] - m[i,:]| tiles: split between VectorE (tensor_scalar subtract
    + sign-bit AND abs, batched) and ScalarE (fused Abs(x + bias)).
  - k-reduction: TensorE matmul against a 0/1 selector stationary, PSUM accum.
  - exp(-d) + j-sum: ScalarE activation with accum_out.
"""

import os
import sys
import types
import numpy as np
import ml_dtypes

N, F, O, K = 512, 512, 128, 8
NCORES = 8
ROWS = N // NCORES            # 64 i-rows per core
NG = 4                        # o-groups of 32
NH = 2                        # k-halves of 4
SPLIT = 52                    # i_loc < SPLIT -> VectorE path, else ScalarE path
assert SPLIT % 4 == 0

_CACHE = {}


def _install_axon_shim():
    """Register the NTFF profile hook module that concourse expects under axon."""
    if 'antenv.axon_hooks' in sys.modules:
        return
    try:
        import antenv
    except ImportError:
        return
    mod = types.ModuleType('antenv.axon_hooks')
    mod._hook = None
    mod.set_axon_ntff_profile_hook = lambda h: setattr(mod, '_hook', h)
    mod.get_axon_ntff_profile_hook = lambda: mod._hook
    sys.modules['antenv.axon_hooks'] = mod
    antenv.axon_hooks = mod
    try:
        from trn_agent_boot.trn_boot import _ntff_profile_via_ctypes
        mod.set_axon_ntff_profile_hook(
            _ntff_profile_via_ctypes('/opt/axon/libaxon_pjrt.so'))
    except Exception:
        pass
    import concourse.bass_utils as bu
    bu.upload_artifacts = lambda tmpdir: tmpdir


def _col_perm():
    """Permutation of T2 columns: new column (g*NH+h)*128 + o_l*4 + k_l maps to
    original column (32g + o_l)*K + 4h + k_l."""
    cols = np.empty(O * K, dtype=np.int64)
    idx = 0
    for g in range(NG):
        for h in range(NH):
            for o_l in range(32):
                for k_l in range(4):
                    cols[idx] = (32 * g + o_l) * K + 4 * h + k_l
                    idx += 1
    return cols


def _build_nc():
    from concourse import bass, mybir, bacc
    from concourse import tile

    dt = mybir.dt
    AF = mybir.ActivationFunctionType
    OP = mybir.AluOpType

    nc = bacc.Bacc("TRN2", target_bir_lowering=False, debug=False)

    xT_d = nc.dram_tensor("xT", [F, N], dt.bfloat16, kind="ExternalInput")
    t2_d = nc.dram_tensor("T2p", [F, O * K], dt.bfloat16, kind="ExternalInput")
    xr_d = nc.dram_tensor("xr", [F, ROWS], dt.bfloat16, kind="ExternalInput")
    sel_d = nc.dram_tensor("sel", [128, 32], dt.bfloat16, kind="ExternalInput")
    out_d = nc.dram_tensor("feats", [128, ROWS], dt.float32, kind="ExternalOutput")

    with tile.TileContext(nc) as tc:
        with tc.tile_pool(name="const", bufs=1) as cp, \
             tc.tile_pool(name="work", bufs=4) as wp, \
             tc.tile_pool(name="pbuild", bufs=1, space="PSUM") as pb, \
             tc.tile_pool(name="pbuild2", bufs=2, space="PSUM") as pb2, \
             tc.tile_pool(name="pd", bufs=5, space="PSUM") as pdp:

            xt = [cp.tile([128, N], dt.bfloat16, tag=f"xt{c}", name=f"xt{c}") for c in range(4)]
            t2 = [cp.tile([128, O * K], dt.bfloat16, tag=f"t2{c}", name=f"t2{c}") for c in range(4)]
            xr = [cp.tile([128, ROWS], dt.bfloat16, tag=f"xr{c}", name=f"xr{c}") for c in range(4)]
            sel = cp.tile([128, 32], dt.bfloat16, tag="sel")
            mt = cp.tile([128, NG * NH * N], dt.bfloat16, tag="mt")       # 8 tiles of [128,512]
            mrb = cp.tile([128, NG * NH * ROWS], dt.bfloat16, tag="mrb")  # bf16-rounded row scalars
            mrf = cp.tile([128, NG * NH * ROWS], dt.float32, tag="mrf")   # +f32 for VectorE
            mrn = cp.tile([128, NG * NH * ROWS], dt.float32, tag="mrn")   # -f32 for ScalarE bias
            feats = cp.tile([128, ROWS], dt.float32, tag="feats")

            for c in range(4):
                nc.sync.dma_start(xt[c][:], xT_d[128 * c:128 * (c + 1), :])
                nc.sync.dma_start(t2[c][:], t2_d[128 * c:128 * (c + 1), :])
                nc.sync.dma_start(xr[c][:], xr_d[128 * c:128 * (c + 1), :])
            nc.sync.dma_start(sel[:], sel_d[:])

            # ---- build m^T tiles (one per (g,h)) and row-scalar columns ----
            pr = pb.tile([128, NG * NH * ROWS], dt.float32, tag="pr")
            for u in range(NG * NH):
                pm = pb2.tile([128, N], dt.float32, tag="pm")
                for c in range(4):
                    lhsT = t2[c][:, 128 * u:128 * (u + 1)]
                    nc.tensor.matmul(pm[:], lhsT, xt[c][:],
                                     start=(c == 0), stop=(c == 3))
                nc.scalar.copy(mt[:, N * u:N * (u + 1)], pm[:])
                for c in range(4):
                    lhsT = t2[c][:, 128 * u:128 * (u + 1)]
                    nc.tensor.matmul(pr[:, ROWS * u:ROWS * (u + 1)], lhsT, xr[c][:],
                                     start=(c == 0), stop=(c == 3))
            nc.scalar.copy(mrb[:], pr[:])
            nc.vector.tensor_copy(mrf[:], mrb[:])          # bf16 -> f32 upcast
            nc.scalar.mul(mrn[:], mrf[:], -1.0)

            # ---- main loop ----
            for g in range(NG):
                for b in range(ROWS // 4):                 # i-batches of 4
                    cbig = wp.tile([128, 8 * N], dt.bfloat16, tag="cbig")
                    dve_batch = (4 * b + 3) < SPLIT
                    for q in range(4):
                        i_loc = 4 * b + q
                        for h in range(NH):
                            u = g * NH + h
                            msl = mt[:, N * u:N * (u + 1)]
                            dst = cbig[:, (q * NH + h) * N:(q * NH + h + 1) * N]
                            if dve_batch:
                                nc.vector.tensor_scalar(
                                    dst, msl,
                                    mrf[:, ROWS * u + i_loc:ROWS * u + i_loc + 1],
                                    None, OP.subtract)
                            else:
                                nc.scalar.activation(
                                    dst, msl, AF.Abs,
                                    bias=mrn[:, ROWS * u + i_loc:ROWS * u + i_loc + 1],
                                    scale=1.0)
                    if dve_batch:
                        cu = cbig[:].bitcast(mybir.dt.uint16)
                        nc.vector.tensor_scalar(cu, cu, 0x7FFF, None, OP.bitwise_and)
                    pd = pdp.tile([128, N], dt.float32, tag="pd")
                    for q in range(4):
                        for h in range(NH):
                            nc.tensor.matmul(
                                pd[32 * q:32 * (q + 1), :], sel[:],
                                cbig[:, (q * NH + h) * N:(q * NH + h + 1) * N],
                                start=(h == 0), stop=(h == 1),
                                tile_position=(0, 32 * q))
                    nc.scalar.activation(pd[:], pd[:], AF.Exp, scale=-1.0,
                                         accum_out=feats[:, g * 16 + b:g * 16 + b + 1])

            nc.sync.dma_start(out_d[:], feats[:])

    nc.compile()
    return nc


def _get_compiled():
    if 'nc' not in _CACHE:
        _install_axon_shim()
        _CACHE['nc'] = _build_nc()
        _CACHE['perm'] = _col_perm()
    return _CACHE['nc'], _CACHE['perm']


def kernel(x: np.ndarray, T: np.ndarray) -> np.ndarray:
    from concourse.bass_utils import run_bass_kernel_spmd

    nc, perm = _get_compiled()

    bf = ml_dtypes.bfloat16
    xT = np.ascontiguousarray(x.T).astype(bf)                       # [F, N]
    t2p = np.ascontiguousarray(T.reshape(F, O * K)[:, perm]).astype(bf)
    selv = (np.arange(128)[:, None] // 4 == np.arange(32)[None, :]).astype(bf)

    in_maps = []
    for c in range(NCORES):
        xr = np.ascontiguousarray(xT[:, ROWS * c:ROWS * (c + 1)])
        in_maps.append({"xT": xT, "T2p": t2p, "xr": xr, "sel": selv})

    trace = bool(int(os.environ.get("MBD_TRACE", "0")))
    res = run_bass_kernel_spmd(nc, in_maps, list(range(NCORES)), trace=trace)
    globals()['LAST_EXEC_NS'] = res.exec_time_ns

    feats = np.empty((N, O), dtype=np.float32)
    for c in range(NCORES):
        fr = res.results[c]["feats"]                                 # [128, 64]
        # partition p = q*32 + o_l ; column = g*16 + b ; i_loc = 4*b + q
        blk = fr.reshape(4, 32, NG, 16).transpose(3, 0, 2, 1).reshape(ROWS, O)
        feats[ROWS * c:ROWS * (c + 1), :] = blk
    return np.concatenate([x.astype(np.float32), feats], axis=1)
